# revision 11
# baseline (speedup 1.0000x reference)
"""GCN 5-layer message-passing kernel for 8 Trainium2 NeuronCores.

Strategy (node-sharded, dst-partitioned, phase/chunk-major gather pipeline):
- Core c owns nodes [c*12500, (c+1)*12500): it aggregates them as dst and
  contributes their transformed features via quarter-AllGathers.
- GCN norm folded: dinv[src] pre-scales the transformed table h2s = dinv*(h@W);
  dinv[dst] scales the epilogue. Self-loops bypass the gather (stT slab).
- Tables are distributed as 4 "quarter chunks": chunk q holds local rows
  [3125q, 3125(q+1)) of every core, AllGathered into a [25000, 128] shared
  table per (layer, q) -> int16 gather indices stay in range.
- Each layer runs in 3 superblock PHASES; within a phase the token stream is
  chunk-major: (phase, chunk, superblock, dst-block), padded to 128-token
  tiles per (block, chunk). Gathers are issued as ~3968-token calls with
  per-(phase,chunk) metadata slabs; aggregation matmuls accumulate into PSUM
  per superblock then into persistent SBUF fp32 accumulators across chunks.
- Epilogues (relu + next-layer transform + own-table writes) run after each
  phase's last chunk; each quarter-AllGather of the NEXT layer's table is
  issued as soon as the epilogues covering that quarter are done, so
  collectives hide behind the remaining gather stream.
- Last layer uses transposed aggregation too (self term from stT), then a
  PE re-transpose feeds the one-hot mean-pool matmul; AllReduce + FC tail.
"""
import os
import numpy as np
import ml_dtypes

N = 100000
E = 1600000
NCORES = 8
NPC = N // NCORES            # 12500 nodes per core
NB = (NPC + 127) // 128      # 98 dst blocks per core
SB_BLOCKS = 4                # dst blocks per superblock
QCH = NPC // 4               # 3125 local rows per quarter chunk
NCHUNK = 4
CHTAB = NCORES * QCH         # 25000 rows per chunk table (< 32768 int16)
GCALL = int(os.environ.get("KGCALL", "1024"))  # tokens per gather call
# single_packet coalesces each engine's whole per-call stream into ONE
# 16KB-max packet: 64 descs x 256B == 16384 caps calls at 1024 tokens.
# Larger calls need per-descriptor packets.
SINGLE_PACKET = GCALL <= 1024
NG = 64                      # graphs
F0 = 128
bf16 = ml_dtypes.bfloat16

# layer i: h2s_i width (transformed source), produced by W_i
FW = [128, 128, 128, 64, 64]   # W5 padded 32->64
TW = 128                       # table row padded to 128 cols bf16 (256B)

# superblock phases; epilogues run at each phase end. Quarter-AllGather q
# needs epilogues of sbs covering rows [3125q, 3125(q+1)) (sb = 512 rows):
# q=0: sbs 0-6, q=1: 6-12, q=2: 12-18, q=3: 18-24.
PHASES = [list(range(0, 10)), list(range(10, 19)), list(range(19, 25))]
PHASE_AGS = [[0], [1, 2], [3]]

_last_results = None
_last_nc = None
_last_in_maps = None


def _superblocks():
    sbs = []
    b = 0
    while b < NB:
        sbs.append(list(range(b, min(b + SB_BLOCKS, NB))))
        b += SB_BLOCKS
    return sbs


def _prep(edge_index):
    """(phase, chunk)-major token layout + per-core gather index/slot arrays."""
    src = edge_index[0]
    dst = edge_index[1]

    sbs = _superblocks()
    sb_of_block = np.zeros(NB, np.int64)
    ph_of_block = np.zeros(NB, np.int64)
    for i, sb in enumerate(sbs):
        for b in sb:
            sb_of_block[b] = i
            ph_of_block[b] = next(p for p, ph in enumerate(PHASES) if i in ph)

    core_edges = []
    counts = np.zeros((NCORES, NB, NCHUNK), np.int64)
    for c in range(NCORES):
        lo = c * NPC
        sel = (dst >= lo) & (dst < lo + NPC)
        es = src[sel]
        ed = dst[sel] - lo
        blk = ed >> 7
        slot = ed & 127
        lq = es % NPC
        chk = lq // QCH
        cidx = (es // NPC) * QCH + (lq - chk * QCH)   # [0, 25000)
        order = np.lexsort((es, blk, chk, ph_of_block[blk]))
        blk, slot, chk, cidx = blk[order], slot[order], chk[order], cidx[order]
        np.add.at(counts[c], (blk, chk), 1)
        core_edges.append((cidx, blk, slot, chk))

    ptiles = (counts.max(axis=0) + 127) // 128     # [NB, NCHUNK] in tiles
    assert ptiles.min() >= 1

    # token layout: (phase, chunk, superblock, block), padded per (blk, chk)
    goff = np.zeros((NB, NCHUNK), np.int64)
    phase_meta = []  # per (p, k): (poff, ntok, [(si, [(b, lb, gt0, nt)])], calls)
    tok = 0
    for p, ph in enumerate(PHASES):
        for k in range(NCHUNK):
            poff = tok
            sblist = []
            for si in ph:
                blocks = []
                for lb, b in enumerate(sbs[si]):
                    goff[b, k] = tok
                    nt = int(ptiles[b, k])
                    blocks.append((b, lb, tok // 128, nt))
                    tok += nt * 128
                sblist.append((si, blocks))
            ntok = tok - poff
            calls = []
            o = 0
            while o < ntok:
                n = min(GCALL, ntok - o)
                calls.append((poff + o, n))
                o += n
            phase_meta.append((p, k, poff, ntok, sblist, calls))
    T = tok

    # per-core token-level arrays in the common layout
    core_data = []
    for c in range(NCORES):
        cidx, blk, slot, chk = core_edges[c]
        gkey = (ph_of_block[blk] * NCHUNK + chk) * NB + blk
        if len(cidx):
            starts = np.r_[0, np.flatnonzero(np.diff(gkey)) + 1]
            runlen = np.diff(np.r_[starts, len(cidx)])
            rank = np.arange(len(cidx)) - np.repeat(starts, runlen)
        else:
            rank = np.zeros(0, np.int64)
        pos = goff[blk, chk] + rank
        idx16 = np.zeros(T, np.int16)              # padding gathers row 0
        slotv = np.full(T, 999.0, np.float32)      # padding never matches
        idx16[pos] = cidx.astype(np.int16)
        slotv[pos] = slot
        gidx_w = np.tile(idx16.reshape(T // 16, 16).T, (8, 1))   # [128, T//16]
        dstloc_w = slotv.reshape(T // 128, 128).T.copy()         # [128, T//128]
        core_data.append((gidx_w, dstloc_w))

    return phase_meta, T, core_data


def _build_program(phase_meta, T):
    import concourse.bass as bass
    import concourse.bacc as bacc
    import concourse.tile as tile
    from concourse import mybir
    dt = mybir.dt
    Alu = mybir.AluOpType
    Act = mybir.ActivationFunctionType

    nc = bacc.Bacc("TRN2", target_bir_lowering=False, debug=False,
                   num_devices=NCORES, num_swdge_queues=4)

    sbs = _superblocks()
    # max tokens of any (phase, chunk) stream, for metadata slab sizing
    max_ptok = max(m[3] for m in phase_meta)

    # ---- IO ----
    xT_d = nc.dram_tensor("xT", [128, NPC], dt.float32, kind="ExternalInput")
    dre_d = nc.dram_tensor("dinvrep", [128, NPC], dt.float32, kind="ExternalInput")
    gidx_d = nc.dram_tensor("gidx", [128, T // 16], dt.int16, kind="ExternalInput")
    dloc_d = nc.dram_tensor("dloc", [128, T // 128], dt.float32, kind="ExternalInput")
    bloc_d = nc.dram_tensor("batchloc", [128, NB], dt.float32, kind="ExternalInput")
    iota_d = nc.dram_tensor("iota", [128, 128], dt.float32, kind="ExternalInput")
    iog_d = nc.dram_tensor("iota64", [128, NG], dt.float32, kind="ExternalInput")
    eye_d = nc.dram_tensor("eye64", [64, 64], dt.float32, kind="ExternalInput")
    W_d = [nc.dram_tensor(f"W{i+1}", [128 if i == 0 else FW[i - 1], FW[i]],
                          dt.float32, kind="ExternalInput") for i in range(5)]
    bias_d = [nc.dram_tensor(f"b{i+1}", [128, 1], dt.float32, kind="ExternalInput")
              for i in range(5)]
    wfc_d = nc.dram_tensor("Wfc", [32, 10], dt.float32, kind="ExternalInput")
    invc_d = nc.dram_tensor("invcrep", [32, NG], dt.float32, kind="ExternalInput")
    bfc_d = nc.dram_tensor("bfcrep", [NG, 10], dt.float32, kind="ExternalInput")
    out_d = nc.dram_tensor("out", [NG, 10], dt.float32, kind="ExternalOutput")

    RG = [list(range(NCORES))]

    with tile.TileContext(nc) as tc:
        with tc.tile_pool(name="cst", bufs=1) as cst, \
             tc.tile_pool(name="sb", bufs=2) as sbp, \
             tc.tile_pool(name="gp", bufs=3) as gpp, \
             tc.tile_pool(name="mt", bufs=2) as mtp, \
             tc.tile_pool(name="acc", bufs=1) as accp, \
             tc.tile_pool(name="ps", bufs=2, space="PSUM") as ps, \
             tc.tile_pool(name="ps2", bufs=1, space="PSUM") as ps2, \
             tc.tile_pool(name="stp", bufs=1, space="PSUM") as stp, \
             tc.tile_pool(name="psp", bufs=1, space="PSUM") as psp, \
             tc.tile_pool(name="dram", bufs=1, space="DRAM") as dram:

            # ---- constants to SBUF ----
            def cload(name, dten, shape, dtype):
                t = cst.tile(shape, dtype, tag=name, name=name)
                nc.sync.dma_start(out=t[:], in_=dten[:])
                return t
            iota_t = cload("iota", iota_d, [128, 128], dt.float32)
            iog_t = cload("iog", iog_d, [128, NG], dt.float32)
            eye_t = cload("eye", eye_d, [64, 64], dt.float32)
            W_t = [cload(f"W{i}", W_d[i], list(W_d[i].shape), dt.float32)
                   for i in range(5)]
            bias_t = [cload(f"b{i}", bias_d[i], [128, 1], dt.float32)
                      for i in range(5)]
            wfc_t = cload("wfc", wfc_d, [32, 10], dt.float32)
            invc_t = cload("invc", invc_d, [32, NG], dt.float32)
            bfc_t = cload("bfc", bfc_d, [NG, 10], dt.float32)
            bloc_t = cload("bloc", bloc_d, [128, NB], dt.float32)
            # transposed own-table slab [w, node] for local self-loop terms
            stT_t = cst.tile([128, NPC], dt.bfloat16, tag="stT")
            # persistent fp32 accumulators, one per superblock
            acc_t = [accp.tile([128, SB_BLOCKS * 128], dt.float32,
                               tag=f"acc{si}", name=f"acc{si}")
                     for si in range(len(sbs))]

            # ---- DRAM internals ----
            # own tables split per quarter so AllGather deps are exact tiles
            ownq = [[dram.tile([QCH, TW], dt.bfloat16, tag=f"own{i}_{q}",
                               name=f"own{i}_{q}")
                     for q in range(NCHUNK)] for i in range(5)]
            fullq = [[dram.tile([CHTAB, TW], dt.bfloat16, tag=f"full{i}_{q}",
                                name=f"full{i}_{q}", addr_space="Shared")
                      for q in range(NCHUNK)]
                     for i in range(5)]
            ar_i = dram.tile([32, NG], dt.float32, tag="ari")
            ar_o = dram.tile([32, NG], dt.float32, tag="aro")

            psum_pool_t = psp.tile([32, NG], dt.float32, tag="pool")
            gq = [0]  # round-robin SWDGE queue counter

            def own_write(i, st, b, rows, w2):
                """DMA st[:rows,:w2] into own quarter tiles for rows
                [b*128, b*128+rows) of layer-i own table (may straddle)."""
                r0 = b * 128
                off = 0
                while off < rows:
                    q = (r0 + off) // QCH
                    qr0 = (r0 + off) - q * QCH
                    qn = min(rows - off, QCH - qr0)
                    nc.sync.dma_start(
                        out=ownq[i][q][qr0:qr0 + qn, 0:w2],
                        in_=st[off:off + qn, 0:w2])
                    off += qn

            def agq(i, q):
                nc.gpsimd.collective_compute(
                    "AllGather", Alu.bypass, replica_groups=RG,
                    ins=[ownq[i][q].opt()], outs=[fullq[i][q].opt()])

            # ---- layer-1 transform: h2s_0 = dinv * (x @ W1) ----
            tf_agsb = {6: 0, 12: 1, 18: 2, 24: 3}
            for si, sb in enumerate(sbs):
                c0 = sb[0] * 128
                cn = min(NPC, (sb[-1] + 1) * 128) - c0
                xsl = sbp.tile([128, SB_BLOCKS * 128], dt.float32, tag="xsl")
                nc.sync.dma_start(out=xsl[:, :cn], in_=xT_d[:, c0:c0 + cn])
                drs = sbp.tile([128, SB_BLOCKS * 128], dt.float32, tag="drs")
                nc.sync.dma_start(out=drs[:, :cn], in_=dre_d[:, c0:c0 + cn])
                xs2 = sbp.tile([128, SB_BLOCKS * 128], dt.float32, tag="xs2")
                nc.vector.tensor_mul(xs2[:, :cn], xsl[:, :cn], drs[:, :cn])
                # transposed table slab for local self-loop terms
                pT = stp.tile([128, SB_BLOCKS * 128], dt.float32, tag="stT_ps")
                nc.tensor.matmul(pT[:, :cn], W_t[0][:], xs2[:, :cn],
                                 start=True, stop=True)
                nc.vector.tensor_copy(out=stT_t[:, c0:c0 + cn], in_=pT[:, :cn])
                for lb, b in enumerate(sb):
                    rows = min(128, NPC - b * 128)
                    pt = ps2.tile([128, FW[0]], dt.float32, tag="tf")
                    nc.tensor.matmul(pt[:rows, :],
                                     xs2[:, lb * 128:lb * 128 + rows],
                                     W_t[0][:], start=True, stop=True)
                    st = sbp.tile([128, FW[0]], dt.bfloat16, tag="h2st")
                    nc.vector.tensor_copy(out=st[:rows, :], in_=pt[:rows, :])
                    own_write(0, st, b, rows, FW[0])
                if si in tf_agsb:
                    agq(0, tf_agsb[si])

            # ---- layers 1..5 ----
            last_si = len(sbs) - 1
            last_lb = len(sbs[-1]) - 1

            def epilogue(i, si):
                w = FW[i]
                sb = sbs[si]
                c0 = si * SB_BLOCKS * 128
                cn = min(NPC - c0, SB_BLOCKS * 128)
                drs = sbp.tile([128, SB_BLOCKS * 128], dt.float32, tag="drse")
                nc.sync.dma_start(out=drs[:, :cn], in_=dre_d[:, c0:c0 + cn])
                tsum = sbp.tile([128, SB_BLOCKS * 128], dt.float32, tag="tsum")
                nc.vector.tensor_add(tsum[:w, :cn], acc_t[si][:w, :cn],
                                     stT_t[:w, c0:c0 + cn])
                tmp = sbp.tile([128, SB_BLOCKS * 128], dt.float32, tag="tmp")
                nc.vector.tensor_mul(tmp[:w, :cn], tsum[:w, :cn], drs[:w, :cn])
                if i < 4:
                    w2 = FW[i + 1]
                    hT = sbp.tile([128, SB_BLOCKS * 128], dt.float32, tag="hT")
                    nc.scalar.activation(hT[:w, :cn], tmp[:w, :cn], Act.Relu,
                                         bias=bias_t[i][:w, 0:1], scale=1.0)
                    # fold next-layer dinv[src] pre-scale
                    hT2 = sbp.tile([128, SB_BLOCKS * 128], dt.float32, tag="hT2")
                    nc.vector.tensor_mul(hT2[:w, :cn], hT[:w, :cn], drs[:w, :cn])
                    # next-layer transposed self-term slab
                    pT = stp.tile([128, SB_BLOCKS * 128], dt.float32,
                                  tag="stT_ps")
                    nc.tensor.matmul(pT[:w2, :cn], W_t[i + 1][:w, :],
                                     hT2[:w, :cn], start=True, stop=True)
                    nc.vector.tensor_copy(out=stT_t[:w2, c0:c0 + cn],
                                          in_=pT[:w2, :cn])
                    for lb, b in enumerate(sb):
                        rows = min(128, NPC - b * 128)
                        pt = ps2.tile([128, 128], dt.float32, tag="tf")
                        nc.tensor.matmul(
                            pt[:rows, :w2],
                            hT2[:w, lb * 128:lb * 128 + rows],
                            W_t[i + 1][:w, :], start=True, stop=True)
                        st = sbp.tile([128, 128], dt.bfloat16, tag="h2st")
                        nc.vector.tensor_copy(out=st[:rows, :w2],
                                              in_=pt[:rows, :w2])
                        own_write(i + 1, st, b, rows, w2)
                else:
                    # last layer: relu -> re-transpose -> mean-pool matmul
                    h5T = sbp.tile([128, SB_BLOCKS * 128], dt.float32, tag="h5T")
                    nc.scalar.activation(h5T[:w, :cn], tmp[:w, :cn], Act.Relu,
                                         bias=bias_t[4][:w, 0:1], scale=1.0)
                    for lb, b in enumerate(sb):
                        rows = min(128, NPC - b * 128)
                        pr = ps2.tile([128, 64], dt.float32, tag="tr5")
                        nc.tensor.matmul(
                            pr[:rows, :],
                            h5T[:w, lb * 128:lb * 128 + rows],
                            eye_t[:w, :], start=True, stop=True)
                        h516 = sbp.tile([128, 32], dt.bfloat16, tag="h516")
                        if rows < 128:
                            nc.vector.memset(h516[:], 0.0)
                        nc.vector.tensor_copy(out=h516[:rows, :],
                                              in_=pr[:rows, 0:32])
                        B = sbp.tile([128, NG], dt.bfloat16, tag="B")
                        nc.vector.tensor_scalar(
                            out=B[:], in0=iog_t[:], scalar1=bloc_t[:, b:b + 1],
                            scalar2=None, op0=Alu.is_equal)
                        nc.tensor.matmul(
                            psum_pool_t[:], h516[:, 0:32], B[:],
                            start=(si == 0 and lb == 0),
                            stop=(si == last_si and lb == last_lb))

            for i in range(5):
                w = FW[i]
                for (p, k, poff, ntok, sblist, calls) in phase_meta:
                    # metadata slabs for the whole (phase, chunk) stream
                    gix = mtp.tile([128, max_ptok // 16], dt.int16, tag="gix")
                    nc.sync.dma_start(
                        out=gix[:, :ntok // 16],
                        in_=gidx_d[:, poff // 16:(poff + ntok) // 16])
                    dsl = mtp.tile([128, max_ptok // 128], dt.float32,
                                   tag="dsl")
                    nc.sync.dma_start(
                        out=dsl[:, :ntok // 128],
                        in_=dloc_d[:, poff // 128:(poff + ntok) // 128])

                    call_tiles = {}
                    emitted = [0]

                    def emit_call(ci, i=i, k=k, poff=poff, calls=calls,
                                  call_tiles=call_tiles, gix=gix, dsl=dsl):
                        coff, cn_ = calls[ci]
                        nt_c = cn_ // 128
                        lo = coff - poff
                        msg = gpp.tile([128, GCALL // 128, TW], dt.bfloat16,
                                       tag="msg")
                        S = gpp.tile([128, GCALL // 128, 128], dt.bfloat16,
                                     tag="S")
                        nc.vector.scalar_tensor_tensor(
                            out=S[:, :nt_c, :],
                            in0=iota_t[:].unsqueeze(1)
                            .broadcast_to([128, nt_c, 128]),
                            scalar=0.0,
                            in1=dsl[:, lo // 128:lo // 128 + nt_c].unsqueeze(2)
                            .broadcast_to([128, nt_c, 128]),
                            op0=Alu.bypass, op1=Alu.is_equal)
                        nc.gpsimd.dma_gather(
                            out_ap=msg[:, 0:nt_c, :],
                            in_ap=fullq[i][k][:],
                            idxs_ap=gix[:, lo // 16:(lo + cn_) // 16],
                            num_idxs=cn_, num_idxs_reg=cn_, elem_size=TW,
                            single_packet=SINGLE_PACKET,
                            queue_num=gq[0] % 4)
                        gq[0] += 1
                        call_tiles[ci] = (msg, S)

                    for (si, blocks) in sblist:
                        last_tile = max(gt0 + nt for (_, _, gt0, nt) in blocks) - 1
                        last_call = (last_tile * 128 - poff) // GCALL
                        while emitted[0] <= last_call:
                            emit_call(emitted[0])
                            emitted[0] += 1
                        pa = ps.tile([128, SB_BLOCKS * 128], dt.float32,
                                     tag="agg")
                        for (b, lb, gt0, nt) in blocks:
                            for t in range(gt0, gt0 + nt):
                                ci = (t * 128 - poff) // GCALL
                                ti = t - (poff + ci * GCALL) // 128
                                msg, S = call_tiles[ci]
                                nc.tensor.matmul(
                                    pa[:w, lb * 128:lb * 128 + 128],
                                    msg[:, ti, 0:w], S[:, ti, :],
                                    start=(t == gt0), stop=(t == gt0 + nt - 1))
                        if k == 0:
                            nc.vector.tensor_copy(out=acc_t[si][:w, :],
                                                  in_=pa[:w, :])
                        else:
                            nc.vector.tensor_add(acc_t[si][:w, :],
                                                 acc_t[si][:w, :], pa[:w, :])
                        if k == NCHUNK - 1:
                            epilogue(i, si)
                    if k == NCHUNK - 1 and i < 4:
                        for q in PHASE_AGS[p]:
                            agq(i + 1, q)

            # ---- pooling tail: AllReduce, scale, FC ----
            pl = sbp.tile([32, NG], dt.float32, tag="pl")
            nc.vector.tensor_copy(out=pl[:], in_=psum_pool_t[:])
            nc.sync.dma_start(out=ar_i[:], in_=pl[:])
            nc.gpsimd.collective_compute(
                "AllReduce", Alu.add, replica_groups=RG,
                ins=[ar_i.opt()], outs=[ar_o.opt()])
            pls = sbp.tile([32, NG], dt.float32, tag="pls")
            nc.sync.dma_start(out=pls[:], in_=ar_o[:])
            plsc = sbp.tile([32, NG], dt.float32, tag="plsc")
            nc.vector.tensor_mul(plsc[:], pls[:], invc_t[:])
            pf = psp.tile([NG, 10], dt.float32, tag="fc")
            nc.tensor.matmul(pf[:], plsc[:], wfc_t[:], start=True, stop=True)
            ot = sbp.tile([NG, 10], dt.float32, tag="ot")
            nc.vector.tensor_add(ot[:], pf[:], bfc_t[:])
            nc.sync.dma_start(out=out_d[:], in_=ot[:])

    nc.compile()
    return nc


def kernel(x, edge_index, batch, W1, b1, W2, b2, W3, b3, W4, b4, W5, b5,
           Wfc, bfc):
    global _last_results, _last_nc, _last_in_maps
    from concourse.bass_utils import run_bass_kernel_spmd

    x = np.asarray(x, np.float32)
    edge_index = np.asarray(edge_index, np.int64)
    batch = np.asarray(batch, np.int64)

    dst_all = np.concatenate([edge_index[1], np.arange(N, dtype=np.int64)])
    deg = np.bincount(dst_all, minlength=N).astype(np.float32)
    dinv = np.where(deg > 0, 1.0 / np.sqrt(deg), 0.0).astype(np.float32)

    phase_meta, T, core_data = _prep(edge_index)

    # weights: W5/b5 padded to 64 outputs
    W5p = np.zeros((64, 64), np.float32)
    W5p[:, :32] = np.asarray(W5, np.float32)
    Ws = [np.asarray(W1, np.float32), np.asarray(W2, np.float32),
          np.asarray(W3, np.float32), np.asarray(W4, np.float32), W5p]
    bs = []
    for b_ in (b1, b2, b3, b4, b5):
        bp = np.zeros((128, 1), np.float32)
        v = np.asarray(b_, np.float32).ravel()
        bp[:v.shape[0], 0] = v
        bs.append(bp)
    cnt = np.bincount(batch, minlength=NG).astype(np.float32)
    invc = (1.0 / np.maximum(cnt, 1.0)).astype(np.float32)
    invc_rep = np.broadcast_to(invc, (32, NG)).copy()
    bfc_rep = np.broadcast_to(np.asarray(bfc, np.float32), (NG, 10)).copy()
    iota = np.broadcast_to(np.arange(128, dtype=np.float32), (128, 128)).copy()
    iota64 = np.broadcast_to(np.arange(NG, dtype=np.float32), (128, NG)).copy()
    eye64 = np.eye(64, dtype=np.float32)

    nc = _build_program(phase_meta, T)

    in_maps = []
    for c in range(NCORES):
        gidx_w, dstloc_w = core_data[c]
        lo = c * NPC
        xT = np.ascontiguousarray(x[lo:lo + NPC].T)
        dre = np.broadcast_to(dinv[lo:lo + NPC], (128, NPC)).copy()
        bpad = np.full(NB * 128, 999.0, np.float32)
        bpad[:NPC] = batch[lo:lo + NPC].astype(np.float32)
        bloc = bpad.reshape(NB, 128).T.copy()
        im = {"xT": xT, "gidx": gidx_w, "dloc": dstloc_w, "dinvrep": dre,
              "batchloc": bloc, "iota": iota, "iota64": iota64,
              "eye64": eye64, "Wfc": np.asarray(Wfc, np.float32),
              "invcrep": invc_rep, "bfcrep": bfc_rep}
        for i in range(5):
            im[f"W{i+1}"] = Ws[i]
            im[f"b{i+1}"] = bs[i]
        in_maps.append(im)

    _last_nc = nc
    _last_in_maps = in_maps
    res = run_bass_kernel_spmd(nc, in_maps, core_ids=list(range(NCORES)))
    _last_results = res
    return np.asarray(res.results[0]["out"], np.float32)


# revision 12
# speedup vs baseline: 1.1087x; 1.1087x over previous
"""GCN 5-layer message-passing kernel for 8 Trainium2 NeuronCores.

Strategy (node-sharded, dst-partitioned):
- Core c owns dst nodes [c*12500, (c+1)*12500). Real edges are routed to the
  core owning their dst; self-loops never enter the gather path — their
  contribution dinv^2 * h2s[v] is added locally (stT slab / own-table load).
- GCN norm factors dinv[src]*dinv[dst] are folded: dinv[src] pre-scales the
  transformed feature table h2s = dinv * (h @ W) before the gather; dinv[dst]
  scales the per-superblock epilogue. No per-edge multiplies on device.
- Per layer: each core computes its slice of h2s (transform matmul), cores
  AllGather the full table (Shared scratchpad), then each core gathers rows
  for its edges via GPSIMD dma_gather (single_packet, 4 SWDGE queues
  round-robin — the gather is the kernel bottleneck at ~10ns/row) and
  reduces them per dst block with one-hot segment matmuls in PSUM.
- All tables are bf16 padded to 128 cols (256B gather rows); pad cols hold
  garbage and are never read (matmuls slice [:, :w]).
- Edges are sorted by (superblock, src-chunk, dst-block) and padded to
  128-token tiles so int16 gather indices stay in range (4 chunks of 32768
  rows) and every 128-token matmul tile maps to a single dst block.
- Epilogues are batched per superblock (4 dst blocks = one PSUM bank wide).
- Final global mean-pool via one-hot matmul + AllReduce, then the FC layer.
"""
import os
import numpy as np
import ml_dtypes

KSKIP_GATHER = os.environ.get("KSKIP_GATHER", "0") == "1"
KSKIP_MM = os.environ.get("KSKIP_MM", "0") == "1"
KSKIP_AG = os.environ.get("KSKIP_AG", "0") == "1"

N = 100000
E = 1600000
NCORES = 8
NPC = N // NCORES            # 12500 nodes per core
NB = (NPC + 127) // 128      # 98 dst blocks per core
SB_BLOCKS = 4                # dst blocks per superblock
CHUNK = 32768                # gather src chunk (int16 index range)
NCHUNK = (N + CHUNK - 1) // CHUNK  # 4
NG = 64                      # graphs
F0 = 128
bf16 = ml_dtypes.bfloat16

# layer i: h2s_i width (transformed source), produced by W_i
FW = [128, 128, 128, 64, 64]   # W5 padded 32->64
# all tables bf16, padded to 128 cols (256B gather rows); pad cols hold
# garbage and are never read (matmuls slice [:, :w])
TW = 128

_last_results = None
_last_nc = None
_last_in_maps = None


def _superblocks():
    sbs = []
    b = 0
    while b < NB:
        sbs.append(list(range(b, min(b + SB_BLOCKS, NB))))
        b += SB_BLOCKS
    return sbs


def _prep(x, edge_index, batch, dinv):
    """Build common program structure + per-core data arrays.

    Self-loops are NOT routed through the gather path: their contribution
    dinv[v]^2 * h2s[v] is added locally from the transposed own-table slab
    (stT) kept in SBUF, so only real edges cost gather traffic."""
    src = edge_index[0]
    dst = edge_index[1]

    sbs = _superblocks()
    sb_of_block = np.zeros(NB, np.int64)
    for i, sb in enumerate(sbs):
        for b in sb:
            sb_of_block[b] = i

    # per-core sorted edge arrays and per-(block, chunk) counts
    core_edges = []
    counts = np.zeros((NCORES, NB, NCHUNK), np.int64)
    for c in range(NCORES):
        lo = c * NPC
        sel = (dst >= lo) & (dst < lo + NPC)
        es = src[sel]
        ed = dst[sel] - lo
        blk = ed >> 7
        slot = ed & 127
        chk = es >> 15
        order = np.lexsort((es, blk, chk, sb_of_block[blk]))
        es, blk, slot, chk = es[order], blk[order], slot[order], chk[order]
        np.add.at(counts[c], (blk, chk), 1)
        core_edges.append((es, blk, slot, chk))

    # common padded tile counts per (block, chunk): max over cores
    ptiles = (counts.max(axis=0) + 127) // 128     # [NB, NCHUNK] in tiles

    # traversal order: (sb, chunk, block) -> token offsets
    goff = np.zeros((NB, NCHUNK), np.int64)
    sb_meta = []   # per sb: (tok_off, ntok, [(k, off_k, n_k)], [(b, lb, [(tile_off, ntiles)])])
    tok = 0
    for sb in sbs:
        sb_off = tok
        chunk_runs = []
        block_tiles = {b: [] for b in sb}
        for k in range(NCHUNK):
            k_off = tok
            for b in sb:
                goff[b, k] = tok
                nt = int(ptiles[b, k])
                if nt:
                    block_tiles[b].append(((tok - sb_off) // 128, nt))
                tok += nt * 128
            n_k = tok - k_off
            if n_k:
                chunk_runs.append((k, k_off - sb_off, n_k))
        sb_meta.append((sb_off, tok - sb_off, chunk_runs,
                        [(b, lb, block_tiles[b]) for lb, b in enumerate(sb)]))
    T = tok

    # per-core data arrays in the common layout
    core_data = []
    for c in range(NCORES):
        es, blk, slot, chk = core_edges[c]
        gkey = blk * NCHUNK + chk
        if len(es):
            starts = np.r_[0, np.flatnonzero(np.diff(gkey)) + 1]
            runlen = np.diff(np.r_[starts, len(es)])
            rank = np.arange(len(es)) - np.repeat(starts, runlen)
        else:
            rank = np.zeros(0, np.int64)
        pos = goff[blk, chk] + rank
        idx16 = np.zeros(T, np.int16)
        slotv = np.full(T, 999.0, np.float32)
        idx16[pos] = (es - chk * CHUNK).astype(np.int16)
        slotv[pos] = slot
        gidx_w = np.tile(idx16.reshape(T // 16, 16).T, (8, 1))
        dstloc_w = slotv.reshape(T // 128, 128).T.copy()
        core_data.append((gidx_w, dstloc_w))

    # per-core node-level arrays
    node_data = []
    for c in range(NCORES):
        lo = c * NPC
        xT = np.ascontiguousarray(x[lo:lo + NPC].T)              # [128, NPC]
        dre = np.broadcast_to(dinv[lo:lo + NPC], (128, NPC)).copy()
        dpad = np.ones(NB * 128, np.float32)
        dpad[:NPC] = dinv[lo:lo + NPC]
        dcol = dpad.reshape(NB, 128).T.copy()                    # [128, NB]
        bpad = np.full(NB * 128, 999.0, np.float32)
        bpad[:NPC] = batch[lo:lo + NPC].astype(np.float32)
        bloc = bpad.reshape(NB, 128).T.copy()                    # [128, NB]
        node_data.append((xT, dre, dcol, bloc))

    return sb_meta, T, core_data, node_data


def _build_program(sb_meta, T, repeats=1):
    import concourse.bass as bass
    import concourse.bacc as bacc
    import concourse.tile as tile
    from concourse import mybir
    dt = mybir.dt
    Alu = mybir.AluOpType
    Act = mybir.ActivationFunctionType

    nc = bacc.Bacc("TRN2", target_bir_lowering=False, debug=False,
                   num_devices=NCORES, num_swdge_queues=4)

    # ---- IO ----
    xT_d = nc.dram_tensor("xT", [128, NPC], dt.float32, kind="ExternalInput")
    gidx_d = nc.dram_tensor("gidx", [128, T // 16], dt.int16, kind="ExternalInput")
    dloc_d = nc.dram_tensor("dloc", [128, T // 128], dt.float32, kind="ExternalInput")
    dre_d = nc.dram_tensor("dinvrep", [128, NPC], dt.float32, kind="ExternalInput")
    dcol_d = nc.dram_tensor("dinvcol", [128, NB], dt.float32, kind="ExternalInput")
    bloc_d = nc.dram_tensor("batchloc", [128, NB], dt.float32, kind="ExternalInput")
    iota_d = nc.dram_tensor("iota", [128, 128], dt.float32, kind="ExternalInput")
    iog_d = nc.dram_tensor("iota64", [128, NG], dt.float32, kind="ExternalInput")
    W_d = [nc.dram_tensor(f"W{i+1}", [128 if i == 0 else FW[i - 1], FW[i]],
                          dt.float32, kind="ExternalInput") for i in range(5)]
    bias_d = [nc.dram_tensor(f"b{i+1}", [128, 1], dt.float32, kind="ExternalInput")
              for i in range(4)]
    b5r_d = nc.dram_tensor("b5rep", [128, 64], dt.float32, kind="ExternalInput")
    wfc_d = nc.dram_tensor("Wfc", [32, 10], dt.float32, kind="ExternalInput")
    invc_d = nc.dram_tensor("invcrep", [32, NG], dt.float32, kind="ExternalInput")
    bfc_d = nc.dram_tensor("bfcrep", [NG, 10], dt.float32, kind="ExternalInput")
    out_d = nc.dram_tensor("out", [NG, 10], dt.float32, kind="ExternalOutput")

    RG = [list(range(NCORES))]

    with tile.TileContext(nc) as tc:
        with tc.tile_pool(name="cst", bufs=1) as cst, \
             tc.tile_pool(name="sb", bufs=2) as sbp, \
             tc.tile_pool(name="ps", bufs=2, space="PSUM") as ps, \
             tc.tile_pool(name="ps2", bufs=1, space="PSUM") as ps2, \
             tc.tile_pool(name="psp", bufs=1, space="PSUM") as psp, \
             tc.tile_pool(name="stp", bufs=1, space="PSUM") as stp, \
             tc.tile_pool(name="dram", bufs=1, space="DRAM") as dram:

            # ---- constants to SBUF ----
            def cload(name, dten, shape, dtype):
                t = cst.tile(shape, dtype, tag=name)
                nc.sync.dma_start(out=t[:], in_=dten[:])
                return t
            iota_t = cload("iota", iota_d, [128, 128], dt.float32)
            iog_t = cload("iog", iog_d, [128, NG], dt.float32)
            W_t = [cload(f"W{i}", W_d[i], list(W_d[i].shape), dt.float32)
                   for i in range(5)]
            bias_t = [cload(f"b{i}", bias_d[i], [128, 1], dt.float32)
                      for i in range(4)]
            b5r_t = cload("b5r", b5r_d, [128, 64], dt.float32)
            wfc_t = cload("wfc", wfc_d, [32, 10], dt.float32)
            invc_t = cload("invc", invc_d, [32, NG], dt.float32)
            bfc_t = cload("bfc", bfc_d, [NG, 10], dt.float32)
            dcol_t = cload("dcol", dcol_d, [128, NB], dt.float32)
            bloc_t = cload("bloc", bloc_d, [128, NB], dt.float32)
            dre_t = cload("dre", dre_d, [128, NPC], dt.float32)
            # transposed own-table slab [w, node] for local self-loop terms
            stT_t = cst.tile([128, NPC], dt.bfloat16, tag="stT")

            # ---- DRAM internals ----
            def h2s_tiles(i):
                own = dram.tile([NPC, TW], dt.bfloat16, tag=f"own{i}")
                full = dram.tile([N, TW], dt.bfloat16, tag=f"full{i}",
                                 addr_space="Shared")
                return own, full
            h2s = [h2s_tiles(i) for i in range(5)]
            ar_i = dram.tile([32, NG], dt.float32, tag="ari")
            ar_o = dram.tile([32, NG], dt.float32, tag="aro")

            sbs = _superblocks()
            gq = [0]  # round-robin SWDGE queue counter
            psum_pool_t = psp.tile([32, NG], dt.float32, tag="pool")
            if KSKIP_MM:
                nc.vector.memset(psum_pool_t[:], 0.0)
            for _rep in range(repeats):

                # ---- layer 1 transform: h2s_0 = dinv * (x @ W1), bf16 ----
                for si, sb in enumerate(sbs):
                    c0 = sb[0] * 128
                    cn = min(NPC, (sb[-1] + 1) * 128) - c0
                    full_sb = len(sb) == SB_BLOCKS and cn == SB_BLOCKS * 128
                    xsl = sbp.tile([128, SB_BLOCKS * 128], dt.float32, tag="xsl")
                    nc.sync.dma_start(out=xsl[:, :cn], in_=xT_d[:, c0:c0 + cn])
                    # fold dinv[node] pre-scale into the transform input
                    xs2 = sbp.tile([128, SB_BLOCKS * 128], dt.float32, tag="xs2")
                    nc.vector.tensor_mul(xs2[:, :cn], xsl[:, :cn],
                                         dre_t[:, c0:c0 + cn])
                    # transposed table slab for local self-loop terms
                    pT = stp.tile([128, SB_BLOCKS * 128], dt.float32,
                                  tag="stT_ps")
                    nc.tensor.matmul(pT[:, :cn], W_t[0][:], xs2[:, :cn],
                                     start=True, stop=True)
                    nc.vector.tensor_copy(out=stT_t[:, c0:c0 + cn],
                                          in_=pT[:, :cn])
                    if full_sb:
                        SBW = SB_BLOCKS * 128
                        st4 = sbp.tile([128, SB_BLOCKS, FW[0]], dt.bfloat16,
                                       tag="st4")
                        for lb in range(SB_BLOCKS):
                            pt = ps2.tile([128, FW[0]], dt.float32, tag="tf")
                            nc.tensor.matmul(
                                pt[:], xs2[:, lb * 128:lb * 128 + 128],
                                W_t[0][:], start=True, stop=True)
                            nc.vector.tensor_copy(out=st4[:, lb, :], in_=pt[:])
                        nc.sync.dma_start(
                            out=h2s[0][0][c0:c0 + SBW, :].rearrange(
                                "(j p) w -> p j w", p=128),
                            in_=st4[:])
                    else:
                        for lb, b in enumerate(sb):
                            rows = min(128, NPC - b * 128)
                            pt = ps2.tile([128, FW[0]], dt.float32, tag="tf")
                            nc.tensor.matmul(
                                pt[:], xs2[:, lb * 128:lb * 128 + 128],
                                W_t[0][:], start=True, stop=True)
                            st = sbp.tile([128, FW[0]], dt.bfloat16, tag="h2st")
                            nc.vector.tensor_copy(out=st[:], in_=pt[:])
                            nc.sync.dma_start(
                                out=h2s[0][0][b * 128:b * 128 + rows, :],
                                in_=st[:rows, :])
                if KSKIP_AG:
                    nc.sync.dma_start(out=h2s[0][1][:NPC, :], in_=h2s[0][0][:])
                else:
                    nc.gpsimd.collective_compute(
                        "AllGather", Alu.bypass, replica_groups=RG,
                        ins=[h2s[0][0].opt()], outs=[h2s[0][1].opt()])

                # ---- layers 1..5: gather + segment-matmul + epilogue (+transform) ----
                for i in range(5):
                    w = FW[i]
                    src_full = h2s[i][1]
                    for si, sb in enumerate(sbs):
                        sb_off, ntok, chunk_runs, blocks = sb_meta[si]
                        nt_sb = ntok // 128
                        # metadata slabs
                        gix = sbp.tile([128, ntok // 16], dt.int16, tag="gix")
                        nc.sync.dma_start(
                            out=gix[:], in_=gidx_d[:, sb_off // 16:(sb_off + ntok) // 16])
                        dsl = sbp.tile([128, nt_sb], dt.float32, tag="dsl")
                        nc.sync.dma_start(
                            out=dsl[:], in_=dloc_d[:, sb_off // 128:(sb_off + ntok) // 128])
                        # gathers per chunk
                        msg = sbp.tile([128, nt_sb, TW], dt.bfloat16, tag="msg")
                        GMAX = 1024  # half ring cap = dynamic_dma_scratch_size//16
                        for (k, off_k, n_k) in chunk_runs:
                            rows_k = min(CHUNK, N - k * CHUNK)
                            for p0 in (range(0, n_k, GMAX) if not KSKIP_GATHER else []):
                                pn = min(GMAX, n_k - p0)
                                o = off_k + p0
                                nc.gpsimd.dma_gather(
                                    out_ap=msg[:, o // 128:(o + pn) // 128, :],
                                    in_ap=src_full[k * CHUNK:k * CHUNK + rows_k, :],
                                    idxs_ap=gix[:, o // 16:(o + pn) // 16],
                                    num_idxs=pn, num_idxs_reg=pn, elem_size=TW,
                                    single_packet=True,
                                    queue_num=gq[0] % 4)
                                gq[0] += 1
                        if KSKIP_MM:
                            # keep the transform chain alive with dummy hT
                            c0 = sb[0] * 128
                            for (b, lb, tiles) in blocks:
                                rows = min(128, NPC - b * 128)
                                if i < 4:
                                    w2 = FW[i + 1]
                                    hT = sbp.tile([w, 128], dt.float32, tag="hT")
                                    nc.vector.memset(hT[:], 0.5)
                                    pt = ps2.tile([128, w2], dt.float32, tag="tf")
                                    nc.tensor.matmul(pt[:], hT[:], W_t[i + 1][:],
                                                     start=True, stop=True)
                                    st = sbp.tile([128, w2], dt.bfloat16, tag="h2st")
                                    nc.vector.tensor_scalar(
                                        out=st[:], in0=pt[:],
                                        scalar1=dcol_t[:, b:b + 1],
                                        scalar2=None, op0=Alu.mult)
                                    nc.sync.dma_start(
                                        out=h2s[i + 1][0][b * 128:b * 128 + rows, 0:w2],
                                        in_=st[:rows, :])
                            continue
                        # S build (one wide DVE op)
                        S = sbp.tile([128, nt_sb, 128], dt.bfloat16, tag="S")
                        nc.vector.scalar_tensor_tensor(
                            out=S[:], in0=iota_t[:].unsqueeze(1).broadcast_to([128, nt_sb, 128]),
                            scalar=0.0,
                            in1=dsl[:].unsqueeze(2).broadcast_to([128, nt_sb, 128]),
                            op0=Alu.bypass, op1=Alu.is_equal)
                        mm = msg
                        c0 = sb[0] * 128
                        full_sb = (len(sb) == SB_BLOCKS
                                   and NPC - sb[-1] * 128 >= 128)
                        if i < 4 and full_sb:
                            # --- batched superblock epilogue (i<4) ---
                            SBW = SB_BLOCKS * 128
                            w2 = FW[i + 1]
                            pa = ps.tile([w, SBW], dt.float32, tag="agg")
                            for (b, lb, tiles) in blocks:
                                tlast = tiles[-1][0] + tiles[-1][1] - 1
                                first = True
                                for (toff, ntl) in tiles:
                                    for t in range(toff, toff + ntl):
                                        nc.tensor.matmul(
                                            pa[:, lb * 128:lb * 128 + 128],
                                            mm[:, t, 0:w], S[:, t, :],
                                            start=first, stop=(t == tlast))
                                        first = False
                            # add local self-loop term, then scale by dinv[dst]
                            tsum = sbp.tile([w, SBW], dt.float32, tag="tsum")
                            nc.vector.tensor_add(tsum[:], pa[:],
                                                 stT_t[:w, c0:c0 + SBW])
                            tmp = sbp.tile([w, SBW], dt.float32, tag="tmp")
                            nc.vector.tensor_mul(tmp[:], tsum[:],
                                                 dre_t[:w, c0:c0 + SBW])
                            hT = sbp.tile([w, SBW], dt.float32, tag="hT")
                            nc.scalar.activation(hT[:], tmp[:], Act.Relu,
                                                 bias=bias_t[i][:w, 0:1],
                                                 scale=1.0)
                            # fold next-layer dinv[src] pre-scale into hT
                            hT2 = sbp.tile([w, SBW], dt.float32, tag="hT2")
                            nc.vector.tensor_mul(hT2[:], hT[:],
                                                 dre_t[:w, c0:c0 + SBW])
                            # next-layer transposed table slab (self terms)
                            pT = stp.tile([128, SB_BLOCKS * 128], dt.float32,
                                          tag="stT_ps")
                            nc.tensor.matmul(pT[:w2, :], W_t[i + 1][:w, :],
                                             hT2[:], start=True, stop=True)
                            nc.vector.tensor_copy(out=stT_t[:w2, c0:c0 + SBW],
                                                  in_=pT[:w2, :])
                            st4 = sbp.tile([128, SB_BLOCKS, w2], dt.bfloat16,
                                           tag="st4")
                            for lb in range(SB_BLOCKS):
                                pt = ps2.tile([128, w2], dt.float32, tag="tf")
                                nc.tensor.matmul(
                                    pt[:], hT2[:, lb * 128:lb * 128 + 128],
                                    W_t[i + 1][:], start=True, stop=True)
                                nc.vector.tensor_copy(out=st4[:, lb, :], in_=pt[:])
                            nc.sync.dma_start(
                                out=h2s[i + 1][0][c0:c0 + SBW, 0:w2].rearrange(
                                    "(j p) w -> p j w", p=128),
                                in_=st4[:])
                            continue
                        if i == 4 and full_sb:
                            # --- batched superblock epilogue (last layer) ---
                            pa4 = ps.tile([128, SB_BLOCKS, w], dt.float32,
                                          tag="agg5")
                            for (b, lb, tiles) in blocks:
                                tlast = tiles[-1][0] + tiles[-1][1] - 1
                                first = True
                                for (toff, ntl) in tiles:
                                    for t in range(toff, toff + ntl):
                                        nc.tensor.matmul(
                                            pa4[:, lb, :], S[:, t, :],
                                            mm[:, t, 0:w],
                                            start=first, stop=(t == tlast))
                                        first = False
                            # add local self-loop term from own table (h2s_4)
                            ld4 = sbp.tile([128, SB_BLOCKS, w], dt.bfloat16,
                                           tag="ld4")
                            nc.sync.dma_start(
                                out=ld4[:],
                                in_=h2s[4][0][c0:c0 + SB_BLOCKS * 128, 0:w]
                                .rearrange("(j p) w -> p j w", p=128))
                            s14 = sbp.tile([128, SB_BLOCKS, w], dt.float32,
                                           tag="s14")
                            nc.vector.tensor_add(s14[:], pa4[:], ld4[:])
                            tmp4 = sbp.tile([128, SB_BLOCKS, w], dt.float32,
                                            tag="tmp54")
                            nc.vector.tensor_mul(
                                tmp4[:], s14[:],
                                dcol_t[:, sb[0]:sb[0] + SB_BLOCKS].unsqueeze(2)
                                .broadcast_to([128, SB_BLOCKS, w]))
                            h5p = sbp.tile([128, SB_BLOCKS, w], dt.float32,
                                           tag="h5p4")
                            nc.vector.tensor_add(
                                h5p[:], tmp4[:],
                                b5r_t[:].unsqueeze(1)
                                .broadcast_to([128, SB_BLOCKS, w]))
                            h516 = sbp.tile([128, SB_BLOCKS, w], dt.bfloat16,
                                            tag="h5164")
                            nc.scalar.activation(h516[:], h5p[:], Act.Relu)
                            B4 = sbp.tile([128, SB_BLOCKS, NG], dt.bfloat16,
                                          tag="B4")
                            nc.vector.scalar_tensor_tensor(
                                out=B4[:],
                                in0=iog_t[:].unsqueeze(1)
                                .broadcast_to([128, SB_BLOCKS, NG]),
                                scalar=0.0,
                                in1=bloc_t[:, sb[0]:sb[0] + SB_BLOCKS]
                                .unsqueeze(2)
                                .broadcast_to([128, SB_BLOCKS, NG]),
                                op0=Alu.bypass, op1=Alu.is_equal)
                            for lb in range(SB_BLOCKS):
                                nc.tensor.matmul(
                                    psum_pool_t[:], h516[:, lb, 0:32],
                                    B4[:, lb, :],
                                    start=(si == 0 and lb == 0), stop=False)
                            continue
                        for (b, lb, tiles) in blocks:
                            rows = min(128, NPC - b * 128)
                            tlast = tiles[-1][0] + tiles[-1][1] - 1
                            if i < 4:
                                # transposed aggregation: psum [w, 128dst]
                                pa = ps.tile([w, 128], dt.float32, tag="agg")
                                first = True
                                for (toff, ntl) in tiles:
                                    for t in range(toff, toff + ntl):
                                        nc.tensor.matmul(
                                            pa[:], mm[:, t, 0:w], S[:, t, :],
                                            start=first, stop=(t == tlast))
                                        first = False
                                # epilogue: hT = relu((pa + self) * dinv + bias)
                                n0 = c0 + lb * 128
                                tsum = sbp.tile([w, 128], dt.float32, tag="tsumf")
                                nc.vector.tensor_add(
                                    tsum[:, :rows], pa[:, :rows],
                                    stT_t[:w, n0:n0 + rows])
                                tmp = sbp.tile([w, 128], dt.float32, tag="tmp")
                                if rows < 128:
                                    nc.vector.memset(tmp[:, rows:], 0.0)
                                nc.vector.tensor_mul(
                                    tmp[:, :rows], tsum[:, :rows],
                                    dre_t[:w, n0:n0 + rows])
                                hT = sbp.tile([w, 128], dt.float32, tag="hT")
                                nc.scalar.activation(hT[:], tmp[:], Act.Relu,
                                                     bias=bias_t[i][:w, 0:1], scale=1.0)
                                # transform: psum2 [128n, w2] = hT.T @ W_{i+1}
                                w2 = FW[i + 1]
                                pt = ps2.tile([128, w2], dt.float32, tag="tf")
                                nc.tensor.matmul(pt[:], hT[:], W_t[i + 1][:],
                                                 start=True, stop=True)
                                st = sbp.tile([128, w2], dt.bfloat16, tag="h2st")
                                nc.vector.tensor_scalar(
                                    out=st[:], in0=pt[:], scalar1=dcol_t[:, b:b + 1],
                                    scalar2=None, op0=Alu.mult)
                                nc.sync.dma_start(
                                    out=h2s[i + 1][0][b * 128:b * 128 + rows, 0:w2],
                                    in_=st[:rows, :])
                                # next-layer transposed slab for this block
                                hT2b = sbp.tile([w, 128], dt.float32, tag="hT2b")
                                nc.vector.tensor_mul(
                                    hT2b[:, :rows], hT[:, :rows],
                                    dre_t[:w, n0:n0 + rows])
                                pTb = stp.tile([128, SB_BLOCKS * 128],
                                               dt.float32, tag="stT_ps")
                                nc.tensor.matmul(
                                    pTb[:w2, :rows], W_t[i + 1][:w, :],
                                    hT2b[:w, :rows], start=True, stop=True)
                                nc.vector.tensor_copy(
                                    out=stT_t[:w2, n0:n0 + rows],
                                    in_=pTb[:w2, :rows])
                            else:
                                # normal aggregation: psum [128dst, 64]
                                pa = ps.tile([128, w], dt.float32, tag="agg")
                                first = True
                                for (toff, ntl) in tiles:
                                    for t in range(toff, toff + ntl):
                                        nc.tensor.matmul(
                                            pa[:], S[:, t, :], mm[:, t, 0:w],
                                            start=first, stop=(t == tlast))
                                        first = False
                                # add local self-loop term from own table
                                ld = sbp.tile([128, w], dt.bfloat16, tag="ld5")
                                if rows < 128:
                                    nc.vector.memset(ld[:], 0.0)
                                nc.sync.dma_start(
                                    out=ld[:rows, :],
                                    in_=h2s[4][0][b * 128:b * 128 + rows, 0:w])
                                s1 = sbp.tile([128, w], dt.float32, tag="s15")
                                nc.vector.tensor_add(s1[:], pa[:], ld[:])
                                # epilogue: h5 = relu(s1 * dinvcol + b5rep)
                                tmp = sbp.tile([128, w], dt.float32, tag="tmp5")
                                nc.vector.scalar_tensor_tensor(
                                    out=tmp[:], in0=s1[:], scalar=dcol_t[:, b:b + 1],
                                    in1=b5r_t[:], op0=Alu.mult, op1=Alu.add)
                                h5 = sbp.tile([128, w], dt.float32, tag="h5")
                                nc.scalar.activation(h5[:], tmp[:], Act.Relu)
                                # pooling: psum_pool [32, NG] += h5[:, :32].T @ B
                                h516 = sbp.tile([128, w], dt.bfloat16, tag="h516")
                                nc.vector.tensor_copy(out=h516[:], in_=h5[:])
                                B = sbp.tile([128, NG], dt.bfloat16, tag="B")
                                nc.vector.tensor_scalar(
                                    out=B[:], in0=iog_t[:], scalar1=bloc_t[:, b:b + 1],
                                    scalar2=None, op0=Alu.is_equal)
                                nc.tensor.matmul(
                                    psum_pool_t[:], h516[:, 0:32], B[:],
                                    start=(si == 0 and lb == 0),
                                    stop=(si == len(sbs) - 1 and lb == len(sb) - 1))
                    if i < 4:
                        if KSKIP_AG:
                            nc.sync.dma_start(out=h2s[i + 1][1][:NPC, :],
                                              in_=h2s[i + 1][0][:])
                        else:
                            nc.gpsimd.collective_compute(
                                "AllGather", Alu.bypass, replica_groups=RG,
                                ins=[h2s[i + 1][0].opt()], outs=[h2s[i + 1][1].opt()])

                # ---- pooling tail: AllReduce, scale, FC ----
                pl = sbp.tile([32, NG], dt.float32, tag="pl")
                nc.vector.tensor_copy(out=pl[:], in_=psum_pool_t[:])
                nc.sync.dma_start(out=ar_i[:], in_=pl[:])
                nc.gpsimd.collective_compute(
                    "AllReduce", Alu.add, replica_groups=RG,
                    ins=[ar_i.opt()], outs=[ar_o.opt()])
                pls = sbp.tile([32, NG], dt.float32, tag="pls")
                nc.sync.dma_start(out=pls[:], in_=ar_o[:])
                plsc = sbp.tile([32, NG], dt.float32, tag="plsc")
                nc.vector.tensor_mul(plsc[:], pls[:], invc_t[:])
                pf = psp.tile([NG, 10], dt.float32, tag="fc")
                nc.tensor.matmul(pf[:], plsc[:], wfc_t[:], start=True, stop=True)
                ot = sbp.tile([NG, 10], dt.float32, tag="ot")
                nc.vector.tensor_add(ot[:], pf[:], bfc_t[:])
                nc.sync.dma_start(out=out_d[:], in_=ot[:])

    nc.compile()
    return nc


def kernel(x, edge_index, batch, W1, b1, W2, b2, W3, b3, W4, b4, W5, b5,
           Wfc, bfc):
    global _last_results, _last_nc, _last_in_maps
    from concourse.bass_utils import run_bass_kernel_spmd

    x = np.asarray(x, np.float32)
    edge_index = np.asarray(edge_index, np.int64)
    batch = np.asarray(batch, np.int64)

    src = np.concatenate([edge_index[0], np.arange(N, dtype=np.int64)])
    dst = np.concatenate([edge_index[1], np.arange(N, dtype=np.int64)])
    deg = np.bincount(dst, minlength=N).astype(np.float32)
    dinv = np.where(deg > 0, 1.0 / np.sqrt(deg), 0.0).astype(np.float32)

    sb_meta, T, core_data, node_data = _prep(x, edge_index, batch, dinv)

    # weights: W5/b5 padded to 64 outputs
    W5p = np.zeros((64, 64), np.float32)
    W5p[:, :32] = np.asarray(W5, np.float32)
    b5p = np.zeros(64, np.float32)
    b5p[:32] = np.asarray(b5, np.float32)
    Ws = [np.asarray(W1, np.float32), np.asarray(W2, np.float32),
          np.asarray(W3, np.float32), np.asarray(W4, np.float32), W5p]
    bs = []
    for b_ in (b1, b2, b3, b4):
        bp = np.zeros((128, 1), np.float32)
        v = np.asarray(b_, np.float32).ravel()
        bp[:v.shape[0], 0] = v
        bs.append(bp)
    b5rep = np.broadcast_to(b5p, (128, 64)).copy()
    cnt = np.bincount(batch, minlength=NG).astype(np.float32)
    invc = (1.0 / np.maximum(cnt, 1.0)).astype(np.float32)
    invc_rep = np.broadcast_to(invc, (32, NG)).copy()
    bfc_rep = np.broadcast_to(np.asarray(bfc, np.float32), (NG, 10)).copy()
    iota = np.broadcast_to(np.arange(128, dtype=np.float32), (128, 128)).copy()
    iota64 = np.broadcast_to(np.arange(NG, dtype=np.float32), (128, NG)).copy()

    nc = _build_program(sb_meta, T, repeats=int(os.environ.get('KREPEATS', '1')))

    in_maps = []
    for c in range(NCORES):
        gidx_w, dstloc_w = core_data[c]
        xT, dre, dcol, bloc = node_data[c]
        im = {"xT": xT, "gidx": gidx_w, "dloc": dstloc_w, "dinvrep": dre,
              "dinvcol": dcol, "batchloc": bloc, "iota": iota, "iota64": iota64,
              "b5rep": b5rep, "Wfc": np.asarray(Wfc, np.float32),
              "invcrep": invc_rep, "bfcrep": bfc_rep}
        for i in range(5):
            im[f"W{i+1}"] = Ws[i]
        for i in range(4):
            im[f"b{i+1}"] = bs[i]
        in_maps.append(im)

    _last_nc = nc
    _last_in_maps = in_maps
    res = run_bass_kernel_spmd(nc, in_maps, core_ids=list(range(NCORES)))
    _last_results = res
    return np.asarray(res.results[0]["out"], np.float32)

